# revision 15
# baseline (speedup 1.0000x reference)
"""Linearized attention Trainium2 kernel (v3).

Reference computation per batch b (C=64 channels, H=W=256, N=65536 pixels,
2 heads x 32 head-dim):
    qkv   = qkv_w @ x                      # per-pixel 1x1 conv
    q,k,v = split(qkv); phi(t) = elu(t)+1
    KV    = phi(k) @ v.T  (per head, contract over pixels)   # [32, 32]
    out_h = KV.T @ phi(q) (per head)
    y     = proj_w @ out_h
Sharding: data-parallel over batch, 1 batch per NeuronCore (8 cores).

v3 layout/scheduling notes (changes vs the original baseline):
- x is host-repacked into "slab" order: per channel, tile-major columns
  [A-half 512 | B-half 512] per tile, so slab loads are single DMAs with
  16 KiB contiguous per-partition lines (64 descr/DMA instead of 128
  small ones, and 8 DMAs instead of 64).
- y is stored as bf16 (host casts back to f32), batched 8 tiles per DMA
  on the *scalar* (ACT) HWDGE ring while x loads use the sync ring.
- Pass-1 PSUM tile is [q(512) | kT_A vT_A (512) | kT_B vT_B (512)]; each
  128-pixel-chunk matmul writes kT/vT 64-col groups at stride 256 so
  every matmul output stays inside one PSUM bank, while phi(k)T and vT
  land in clean contiguous chunk-major SBUF buffers.
- phi = max(x,0) + min(exp(x),1) exactly; exp on ACT (2 ops/tile), the
  min runs on the otherwise-idle GPSIMD engine, the two fused
  (max 0)+add ops and the vT cast run on DVE.
- KV^T accumulates over all 512 chunks in one PSUM bank (as baseline).
"""

import sys

if "/opt/trn_rl_repo" not in sys.path:
    sys.path.insert(0, "/opt/trn_rl_repo")

import numpy as np
import ml_dtypes

import concourse.bacc as bacc
import concourse.bass as bass
import concourse.mybir as mybir
import concourse.tile as tile
from concourse.bass_utils import run_bass_kernel_spmd

AF = mybir.ActivationFunctionType
ALU = mybir.AluOpType
F32 = mybir.dt.float32
BF16 = mybir.dt.bfloat16

B, C, H, W = 8, 64, 256, 256
N = H * W            # pixels per batch
HALF = N // 2
NT = 512             # pixels per half-image per tile
NTILES = HALF // NT  # 64
CHUNKS = 8           # 128-pixel transposed chunks per tile
GT = 4               # tiles per DMA slab
NSLAB = NTILES // GT  # 16

_cached = None


def _build():
    nc = bacc.Bacc("TRN2", target_bir_lowering=False, debug=False)

    # x packed tile-major: col block t*1024+[0:512] = A pixels of tile t,
    # [512:1024] = B pixels.
    x_d = nc.dram_tensor("x", [C, N], BF16, kind="ExternalInput")
    # dual-row copy of x: partition p<64 = ch p of A pixels, p>=64 = ch of B
    x2_d = nc.dram_tensor("x2", [128, HALF], BF16, kind="ExternalInput")
    wq_d = nc.dram_tensor("wq", [128, 64], BF16, kind="ExternalInput")
    wkv_d = nc.dram_tensor("wkv", [64, 128], BF16, kind="ExternalInput")
    pj_d = nc.dram_tensor("pj", [64, 64], BF16, kind="ExternalInput")
    # y packed: partition p = (half, out_ch), col = t*512 + pix
    y_d = nc.dram_tensor("y", [128, HALF], BF16, kind="ExternalOutput")

    with tile.TileContext(nc) as tc:
        with (
            tc.tile_pool(name="persist", bufs=1) as persist,
            tc.tile_pool(name="stash", bufs=1) as stash_pool,
        ):
            wq = persist.tile([128, 64], BF16)
            wkv = persist.tile([64, 128], BF16)
            pj = persist.tile([64, 64], BF16)
            w2 = persist.tile([128, 64], BF16)
            kvbd = persist.tile([64, 64], BF16)
            nc.sync.dma_start(wq[:], wq_d.ap())
            nc.sync.dma_start(wkv[:], wkv_d.ap())
            nc.sync.dma_start(pj[:], pj_d.ap())
            nc.gpsimd.memset(kvbd[:], 0.0)

            # phi(q) stash: c-major, half A rows 0:64, half B rows 64:128
            stash = stash_pool.tile([128, HALF], BF16)

            # HAM warmup: dummy back-to-back matmuls on a zeroed scratch
            # tile while the first x slab loads, so real matmuls start at
            # 2.4 GHz instead of 1.2 GHz.
            with (
                tc.tile_pool(name="warm", bufs=1) as warm_pool,
                tc.tile_pool(name="warmps", bufs=1, space="PSUM") as warmps_pool,
            ):
                wsb = warm_pool.tile([64, 512], BF16)
                wps = warmps_pool.tile([64, 512], F32)
                nc.vector.memset(wsb[:], 0.0)
                for _ in range(18):
                    nc.tensor.matmul(
                        wps[:], wsb[:, 0:64], wsb[:], start=True, stop=True
                    )

            # ---------------- pass 1 ----------------
            with (
                tc.tile_pool(name="xs", bufs=2) as xs_pool,
                tc.tile_pool(name="p1sb", bufs=3) as p1sb,
                tc.tile_pool(name="pqk", bufs=2, space="PSUM") as pqk_pool,
                tc.tile_pool(name="pv", bufs=2, space="PSUM") as pv_pool,
                tc.tile_pool(name="kvacc", bufs=1, space="PSUM") as kvacc_pool,
            ):
                kvacc = kvacc_pool.tile([64, 64], F32, tag="kvacc")

                xs = x2s = None
                for t in range(NTILES):
                    g, ti = divmod(t, GT)
                    if ti == 0:
                        # one DMA per 4-tile slab; 8 KiB contiguous lines
                        xs = xs_pool.tile([64, GT * 2 * NT], BF16, tag="xs")
                        nc.sync.dma_start(
                            xs[:],
                            bass.AP(x_d, g * GT * 2 * NT, [[N, 64], [1, GT * 2 * NT]]),
                        )
                        x2s = xs_pool.tile([128, GT * NT], BF16, tag="x2s")
                        nc.sync.dma_start(
                            x2s[:],
                            bass.AP(x2_d, g * GT * NT, [[HALF, 128], [1, GT * NT]]),
                        )
                    xt = xs[:, ti * 2 * NT:(ti + 1) * 2 * NT]  # [64, 1024]
                    x2t = x2s[:, ti * NT:(ti + 1) * NT]        # [128, 512]

                    # PSUM: pqk = [q(512) | kT(512, chunk-major)], pv = vT;
                    # all matmul outs contiguous (strided PSUM MM writes
                    # measured 2.4x slower)
                    pqk = pqk_pool.tile([128, 2 * NT], F32)
                    pv = pv_pool.tile([128, NT], F32)

                    # q (c-major): halves packed on partitions; emitted first
                    # so exp can start while the k/v chunk matmuls still run
                    nc.tensor.matmul(
                        pqk[0:64, 0:NT], wq[0:64, :], x2t[0:64, :],
                        start=True, stop=True,
                    )
                    nc.tensor.matmul(
                        pqk[64:128, 0:NT], wq[64:128, :], x2t[64:128, :],
                        start=True, stop=True, tile_position=(64, 64),
                    )
                    # kT then vT chunks (x-chunk stationary, shared between
                    # the two moving operands); contiguous chunk-major outs
                    for s in range(CHUNKS):
                        nc.tensor.matmul(
                            pqk[:, NT + s * 64:NT + (s + 1) * 64],
                            xt[:, s * 128:(s + 1) * 128],
                            wkv[:, 0:64],
                            start=True, stop=True,
                        )
                    for s in range(CHUNKS):
                        nc.tensor.matmul(
                            pv[:, s * 64:(s + 1) * 64],
                            xt[:, s * 128:(s + 1) * 128],
                            wkv[:, 64:128],
                            start=True, stop=True,
                        )

                    # phi = max(x,0) + min(exp(x),1) for q and kT; all APs 2D
                    eqk = p1sb.tile([128, 2 * NT], BF16, tag="eqk")
                    nc.scalar.activation(eqk[:], pqk[:], AF.Exp)
                    mine = p1sb.tile([128, 2 * NT], BF16, tag="mine")
                    nc.vector.tensor_scalar_min(mine[:], eqk[:], 1.0)
                    kphiT = p1sb.tile([128, NT], BF16, tag="kphiT")
                    nc.vector.scalar_tensor_tensor(
                        kphiT[:], pqk[:, NT:2 * NT], 0.0, mine[:, NT:2 * NT],
                        op0=ALU.max, op1=ALU.add,
                    )
                    nc.vector.scalar_tensor_tensor(
                        stash[:, bass.ts(t, NT)], pqk[:, 0:NT], 0.0,
                        mine[:, 0:NT],
                        op0=ALU.max, op1=ALU.add,
                    )
                    vt = p1sb.tile([128, NT], BF16, tag="vt")
                    nc.scalar.copy(vt[:], pv[:])

                    # KV^T accumulation (both heads; off-diag ignored later)
                    for s in range(CHUNKS):
                        nc.tensor.matmul(
                            kvacc[:],
                            vt[:, bass.ts(s, 64)],
                            kphiT[:, bass.ts(s, 64)],
                            start=(t == 0 and s == 0),
                            stop=(t == NTILES - 1 and s == CHUNKS - 1),
                            skip_group_check=True,
                        )

                # block-diagonal KV^T (cross-head garbage dropped)
                for h0, h1 in ((0, 32), (32, 64)):
                    nc.vector.tensor_copy(
                        kvbd[h0:h1, h0:h1], kvacc[h0:h1, h0:h1]
                    )

            # ---------------- boundary: W2 = blockdiag(KV) @ proj.T ------
            with tc.tile_pool(name="bps", bufs=1, space="PSUM") as bps:
                w2ps = bps.tile([64, 64], F32)
                nc.tensor.matmul(w2ps[:], kvbd[:], pj[:], start=True, stop=True)
                nc.vector.tensor_copy(w2[0:64, :], w2ps[:])
                nc.vector.tensor_copy(w2[64:128, :], w2ps[:])

            # ---------------- pass 2: y = W2.T @ phi(q) ----------------
            with (
                tc.tile_pool(name="p2sb", bufs=3) as p2sb,
                tc.tile_pool(name="yps", bufs=3, space="PSUM") as yps_pool,
            ):
                GT2 = 8
                ysb = None
                for t in range(NTILES):
                    g, ti = divmod(t, GT2)
                    if ti == 0:
                        ysb = p2sb.tile([128, GT2 * NT], BF16, tag="ysb")
                    cs = bass.ts(t, NT)
                    y_ps = yps_pool.tile([128, NT], F32)
                    nc.tensor.matmul(
                        y_ps[0:64, :], w2[0:64, :], stash[0:64, cs],
                        start=True, stop=True,
                    )
                    nc.tensor.matmul(
                        y_ps[64:128, :], w2[64:128, :], stash[64:128, cs],
                        start=True, stop=True, tile_position=(64, 64),
                    )
                    # split the PSUM->SBUF cast between ACT and DVE;
                    # tensor_scalar_add is a 1-input op eligible for the
                    # DVE 2x PSUM read mode (tensor_copy measured 1x)
                    if ti % 2 == 0:
                        nc.scalar.copy(ysb[:, bass.ts(ti, NT)], y_ps[:])
                    else:
                        nc.vector.tensor_scalar_add(
                            ysb[:, bass.ts(ti, NT)], y_ps[:], 0.0
                        )
                    if ti == GT2 - 1:
                        # batched bf16 store, 8 KiB lines, sync HWDGE ring
                        nc.sync.dma_start(
                            bass.AP(y_d, g * GT2 * NT, [[HALF, 128], [1, GT2 * NT]]),
                            ysb[:],
                        )

    nc.compile()
    return nc


def _get_nc():
    global _cached
    if _cached is None:
        _cached = _build()
    return _cached


def _prep_weights(qkv_w, proj_w):
    wq1 = qkv_w[0:64].T
    wq = np.ascontiguousarray(
        np.concatenate([wq1, wq1], axis=0)
    ).astype(ml_dtypes.bfloat16)
    wkT = qkv_w[64:128].T
    wvT = qkv_w[128:192].T
    wkv = np.ascontiguousarray(
        np.concatenate([wkT, wvT], axis=1)
    ).astype(ml_dtypes.bfloat16)
    pj = np.ascontiguousarray(proj_w.T).astype(ml_dtypes.bfloat16)
    return wq, wkv, pj


def _pack_x(xb):
    # [64, N] -> tile-major [A512 | B512] per tile
    xr = xb.reshape(C, 2, NTILES, NT)           # [c, half, tile, col]
    xp = np.transpose(xr, (0, 2, 1, 3))          # [c, tile, half, col]
    return np.ascontiguousarray(xp.reshape(C, N))


def _pack_x2(xb):
    # [64, N] -> [128, HALF]: partition = half*64 + ch, col = tile*512 + pix
    xr = xb.reshape(C, 2, HALF)                  # [c, half, col]
    xp = np.transpose(xr, (1, 0, 2))             # [half, c, col]
    return np.ascontiguousarray(xp.reshape(128, HALF))


def _unpack_y(y_dev):
    # y_dev [128, HALF] bf16: part p = (half, och), col = tile*512 + pix
    yr = np.asarray(y_dev, dtype=np.float32).reshape(2, 64, HALF)
    return np.transpose(yr, (1, 0, 2)).reshape(C, H, W)


def run(x, qkv_w, proj_w, trace=False):
    nc = _get_nc()
    wq, wkv, pj = _prep_weights(np.asarray(qkv_w), np.asarray(proj_w))
    x = np.asarray(x)
    in_maps = [
        {
            "x": _pack_x(
                x[b].reshape(C, N).astype(ml_dtypes.bfloat16)
            ),
            "x2": _pack_x2(
                x[b].reshape(C, N).astype(ml_dtypes.bfloat16)
            ),
            "wq": wq,
            "wkv": wkv,
            "pj": pj,
        }
        for b in range(B)
    ]
    res = run_bass_kernel_spmd(nc, in_maps, core_ids=list(range(B)), trace=trace)
    out = np.stack([_unpack_y(res.results[b]["y"]) for b in range(B)])
    return out.astype(np.float32), res


def kernel(x, qkv_w, proj_w):
    out, _ = run(x, qkv_w, proj_w, trace=False)
    return out


# revision 16
# speedup vs baseline: 1.0547x; 1.0547x over previous
"""Linearized attention Trainium2 kernel (v3).

Reference computation per batch b (C=64 channels, H=W=256, N=65536 pixels,
2 heads x 32 head-dim):
    qkv   = qkv_w @ x                      # per-pixel 1x1 conv
    q,k,v = split(qkv); phi(t) = elu(t)+1
    KV    = phi(k) @ v.T  (per head, contract over pixels)   # [32, 32]
    out_h = KV.T @ phi(q) (per head)
    y     = proj_w @ out_h
Sharding: data-parallel over batch, 1 batch per NeuronCore (8 cores).

v3 layout/scheduling notes (changes vs the original baseline):
- x is host-repacked into "slab" order: per channel, tile-major columns
  [A-half 512 | B-half 512] per tile, so slab loads are single DMAs with
  16 KiB contiguous per-partition lines (64 descr/DMA instead of 128
  small ones, and 8 DMAs instead of 64).
- y is stored as bf16 (host casts back to f32), batched 8 tiles per DMA
  on the *scalar* (ACT) HWDGE ring while x loads use the sync ring.
- Pass-1 PSUM tile is [q(512) | kT_A vT_A (512) | kT_B vT_B (512)]; each
  128-pixel-chunk matmul writes kT/vT 64-col groups at stride 256 so
  every matmul output stays inside one PSUM bank, while phi(k)T and vT
  land in clean contiguous chunk-major SBUF buffers.
- phi = max(x,0) + min(exp(x),1) exactly; exp on ACT (2 ops/tile), the
  min runs on the otherwise-idle GPSIMD engine, the two fused
  (max 0)+add ops and the vT cast run on DVE.
- KV^T accumulates over all 512 chunks in one PSUM bank (as baseline).
"""

import sys

if "/opt/trn_rl_repo" not in sys.path:
    sys.path.insert(0, "/opt/trn_rl_repo")

import numpy as np
import ml_dtypes

import concourse.bacc as bacc
import concourse.bass as bass
import concourse.mybir as mybir
import concourse.tile as tile
from concourse.bass_utils import run_bass_kernel_spmd

AF = mybir.ActivationFunctionType
ALU = mybir.AluOpType
F32 = mybir.dt.float32
BF16 = mybir.dt.bfloat16

B, C, H, W = 8, 64, 256, 256
N = H * W            # pixels per batch
HALF = N // 2
NT = 512             # pixels per half-image per tile
NTILES = HALF // NT  # 64
CHUNKS = 8           # 128-pixel transposed chunks per tile
GT = 4               # tiles per DMA slab
NSLAB = NTILES // GT  # 16

_cached = None


def _build():
    nc = bacc.Bacc("TRN2", target_bir_lowering=False, debug=False)

    # x packed tile-major: col block t*1024+[0:512] = A pixels of tile t,
    # [512:1024] = B pixels.
    x_d = nc.dram_tensor("x", [C, N], BF16, kind="ExternalInput")
    wq_d = nc.dram_tensor("wq", [64, 64], BF16, kind="ExternalInput")
    wkv_d = nc.dram_tensor("wkv", [64, 128], BF16, kind="ExternalInput")
    pj_d = nc.dram_tensor("pj", [64, 64], BF16, kind="ExternalInput")
    # y packed: partition p = (half, out_ch), col = t*512 + pix
    y_d = nc.dram_tensor("y", [128, HALF], BF16, kind="ExternalOutput")

    with tile.TileContext(nc) as tc:
        with (
            tc.tile_pool(name="persist", bufs=1) as persist,
            tc.tile_pool(name="stash", bufs=1) as stash_pool,
        ):
            wq = persist.tile([64, 64], BF16)
            wkv = persist.tile([64, 128], BF16)
            pj = persist.tile([64, 64], BF16)
            w2 = persist.tile([128, 64], BF16)
            kvbd = persist.tile([64, 64], BF16)
            nc.scalar.dma_start(wq[:], wq_d.ap())
            nc.scalar.dma_start(wkv[:], wkv_d.ap())
            nc.scalar.dma_start(pj[:], pj_d.ap())
            nc.gpsimd.memset(kvbd[:], 0.0)

            # phi(q) stash: c-major, half A rows 0:64, half B rows 64:128
            stash = stash_pool.tile([128, HALF], BF16)

            # HAM warmup: dummy back-to-back matmuls on a zeroed scratch
            # tile while the first x slab loads, so real matmuls start at
            # 2.4 GHz instead of 1.2 GHz.
            with (
                tc.tile_pool(name="warm", bufs=1) as warm_pool,
                tc.tile_pool(name="warmps", bufs=1, space="PSUM") as warmps_pool,
            ):
                wsb = warm_pool.tile([64, 512], BF16)
                wps = warmps_pool.tile([64, 512], F32)
                nc.vector.memset(wsb[:], 0.0)
                for _ in range(10):
                    nc.tensor.matmul(
                        wps[:], wsb[:, 0:64], wsb[:], start=True, stop=True
                    )

            # ---------------- pass 1 ----------------
            with (
                tc.tile_pool(name="xs", bufs=2) as xs_pool,
                tc.tile_pool(name="p1sb", bufs=3) as p1sb,
                tc.tile_pool(name="pqk", bufs=2, space="PSUM") as pqk_pool,
                tc.tile_pool(name="pv", bufs=2, space="PSUM") as pv_pool,
                tc.tile_pool(name="kvacc", bufs=1, space="PSUM") as kvacc_pool,
            ):
                kvacc = kvacc_pool.tile([64, 64], F32, tag="kvacc")

                xs = None
                for t in range(NTILES):
                    g, ti = divmod(t, GT)
                    if ti == 0:
                        # one DMA per 4-tile slab; 8 KiB contiguous lines
                        xs = xs_pool.tile([64, GT * 2 * NT], BF16, tag="xs")
                        nc.sync.dma_start(
                            xs[:],
                            bass.AP(x_d, g * GT * 2 * NT, [[N, 64], [1, GT * 2 * NT]]),
                        )
                    xt = xs[:, ti * 2 * NT:(ti + 1) * 2 * NT]  # [64, 1024]

                    # PSUM: pqk = [q(512) | kT(512, chunk-major)], pv = vT;
                    # all matmul outs contiguous (strided PSUM MM writes
                    # measured 2.4x slower)
                    pqk = pqk_pool.tile([128, 2 * NT], F32)
                    pv = pv_pool.tile([128, NT], F32)

                    # q (c-major): halves packed on partitions; emitted first
                    # so exp can start while the k/v chunk matmuls still run
                    nc.tensor.matmul(
                        pqk[0:64, 0:NT], wq[:], xt[:, 0:NT],
                        start=True, stop=True,
                    )
                    nc.tensor.matmul(
                        pqk[64:128, 0:NT], wq[:], xt[:, NT:2 * NT],
                        start=True, stop=True, tile_position=(0, 64),
                    )
                    # kT then vT chunks (x-chunk stationary, shared between
                    # the two moving operands); contiguous chunk-major outs
                    for s in range(CHUNKS):
                        nc.tensor.matmul(
                            pqk[:, NT + s * 64:NT + (s + 1) * 64],
                            xt[:, s * 128:(s + 1) * 128],
                            wkv[:, 0:64],
                            start=True, stop=True,
                        )
                    for s in range(CHUNKS):
                        nc.tensor.matmul(
                            pv[:, s * 64:(s + 1) * 64],
                            xt[:, s * 128:(s + 1) * 128],
                            wkv[:, 64:128],
                            start=True, stop=True,
                        )

                    # phi = max(x,0) + min(exp(x),1) for q and kT; all APs 2D
                    eqk = p1sb.tile([128, 2 * NT], BF16, tag="eqk")
                    nc.scalar.activation(eqk[:], pqk[:], AF.Exp)
                    mine = p1sb.tile([128, 2 * NT], BF16, tag="mine")
                    nc.vector.tensor_scalar_min(mine[:], eqk[:], 1.0)
                    kphiT = p1sb.tile([128, NT], BF16, tag="kphiT")
                    nc.vector.scalar_tensor_tensor(
                        kphiT[:], pqk[:, NT:2 * NT], 0.0, mine[:, NT:2 * NT],
                        op0=ALU.max, op1=ALU.add,
                    )
                    nc.vector.scalar_tensor_tensor(
                        stash[:, bass.ts(t, NT)], pqk[:, 0:NT], 0.0,
                        mine[:, 0:NT],
                        op0=ALU.max, op1=ALU.add,
                    )
                    vt = p1sb.tile([128, NT], BF16, tag="vt")
                    nc.scalar.copy(vt[:], pv[:])

                    # KV^T accumulation, software-pipelined one tile behind
                    # so the PE never stalls on this tile's phi(k)/vT
                    if t > 0:
                        pvt, pkp = prev
                        for s in range(CHUNKS):
                            nc.tensor.matmul(
                                kvacc[:],
                                pvt[:, bass.ts(s, 64)],
                                pkp[:, bass.ts(s, 64)],
                                start=(t == 1 and s == 0),
                                stop=False,
                                skip_group_check=True,
                            )
                    prev = (vt, kphiT)

                # final tile's accumulation
                pvt, pkp = prev
                for s in range(CHUNKS):
                    nc.tensor.matmul(
                        kvacc[:],
                        pvt[:, bass.ts(s, 64)],
                        pkp[:, bass.ts(s, 64)],
                        start=False,
                        stop=(s == CHUNKS - 1),
                        skip_group_check=True,
                    )

                # block-diagonal KV^T (cross-head garbage dropped)
                for h0, h1 in ((0, 32), (32, 64)):
                    nc.vector.tensor_copy(
                        kvbd[h0:h1, h0:h1], kvacc[h0:h1, h0:h1]
                    )

            # ---------------- boundary: W2 = blockdiag(KV) @ proj.T ------
            with tc.tile_pool(name="bps", bufs=1, space="PSUM") as bps:
                w2ps = bps.tile([64, 64], F32)
                nc.tensor.matmul(w2ps[:], kvbd[:], pj[:], start=True, stop=True)
                nc.vector.tensor_copy(w2[0:64, :], w2ps[:])
                nc.vector.tensor_copy(w2[64:128, :], w2ps[:])

            # ---------------- pass 2: y = W2.T @ phi(q) ----------------
            with (
                tc.tile_pool(name="p2sb", bufs=4) as p2sb,
                tc.tile_pool(name="yps", bufs=3, space="PSUM") as yps_pool,
            ):
                GT2 = 8
                ysb = None
                for t in range(NTILES):
                    g, ti = divmod(t, GT2)
                    if ti == 0:
                        ysb = p2sb.tile([128, GT2 * NT], BF16, tag="ysb")
                    cs = bass.ts(t, NT)
                    y_ps = yps_pool.tile([128, NT], F32)
                    nc.tensor.matmul(
                        y_ps[0:64, :], w2[0:64, :], stash[0:64, cs],
                        start=True, stop=True,
                    )
                    nc.tensor.matmul(
                        y_ps[64:128, :], w2[64:128, :], stash[64:128, cs],
                        start=True, stop=True, tile_position=(64, 64),
                    )
                    # split the PSUM->SBUF cast between ACT and DVE;
                    # tensor_scalar_add is a 1-input op eligible for the
                    # DVE 2x PSUM read mode (tensor_copy measured 1x)
                    if ti % 2 == 0:
                        nc.scalar.copy(ysb[:, bass.ts(ti, NT)], y_ps[:])
                    else:
                        nc.vector.tensor_scalar_add(
                            ysb[:, bass.ts(ti, NT)], y_ps[:], 0.0
                        )
                    if ti == GT2 - 1:
                        # batched bf16 store, 8 KiB lines, sync HWDGE ring
                        nc.sync.dma_start(
                            bass.AP(y_d, g * GT2 * NT, [[HALF, 128], [1, GT2 * NT]]),
                            ysb[:],
                        )

    nc.compile()
    return nc


def _get_nc():
    global _cached
    if _cached is None:
        _cached = _build()
    return _cached


def _prep_weights(qkv_w, proj_w):
    wq = np.ascontiguousarray(qkv_w[0:64].T).astype(ml_dtypes.bfloat16)
    wkT = qkv_w[64:128].T
    wvT = qkv_w[128:192].T
    wkv = np.ascontiguousarray(
        np.concatenate([wkT, wvT], axis=1)
    ).astype(ml_dtypes.bfloat16)
    pj = np.ascontiguousarray(proj_w.T).astype(ml_dtypes.bfloat16)
    return wq, wkv, pj


def _pack_x(xb):
    # [64, N] -> tile-major [A512 | B512] per tile
    xr = xb.reshape(C, 2, NTILES, NT)           # [c, half, tile, col]
    xp = np.transpose(xr, (0, 2, 1, 3))          # [c, tile, half, col]
    return np.ascontiguousarray(xp.reshape(C, N))



def _unpack_y(y_dev):
    # y_dev [128, HALF] bf16: part p = (half, och), col = tile*512 + pix
    yr = np.asarray(y_dev, dtype=np.float32).reshape(2, 64, HALF)
    return np.transpose(yr, (1, 0, 2)).reshape(C, H, W)


def run(x, qkv_w, proj_w, trace=False):
    nc = _get_nc()
    wq, wkv, pj = _prep_weights(np.asarray(qkv_w), np.asarray(proj_w))
    x = np.asarray(x)
    in_maps = [
        {
            "x": _pack_x(
                x[b].reshape(C, N).astype(ml_dtypes.bfloat16)
            ),
            "wq": wq,
            "wkv": wkv,
            "pj": pj,
        }
        for b in range(B)
    ]
    res = run_bass_kernel_spmd(nc, in_maps, core_ids=list(range(B)), trace=trace)
    out = np.stack([_unpack_y(res.results[b]["y"]) for b in range(B)])
    return out.astype(np.float32), res


def kernel(x, qkv_w, proj_w):
    out, _ = run(x, qkv_w, proj_w, trace=False)
    return out
